# revision 1
# baseline (speedup 1.0000x reference)
"""CapsuleLayer dynamic-routing kernel for 8x trn2 NeuronCores.

Sharding: input-capsule axis i (2048) split 8 ways (256/core). Routing
softmax over j is core-local; the per-iteration s = sum_i c*u_hat is
partial per core and finished with an 8-core AllReduce. u_hat is never
materialized: each routing pass recomputes the needed contractions from
W directly (contraction over (i,l) or k), keeping PE efficiency high.

Host-side layouts per core (i-shard, IS=256, IL=IS*L=4096):
  XT [IL, B]        x^T               (pass-0 stationary / Y-formation)
  XB [B, IL]        x                 (g-pass elementwise)
  WB [IL, J*K]      W as [(i,l),(j,k)] (pass-0 + s-pass moving)
  W2 [J, K, IL]     W as [j,k,(i,l)]  (t-pass moving)
"""

import numpy as np

B, I_FULL, L = 64, 2048, 16
J, K = 64, 32
NCORES = 8
IS = I_FULL // NCORES          # 256 i per core
IL = IS * L                    # 4096
NCH = IL // 128                # 32 contraction chunks of 128
JK = J * K                     # 2048
EPS = 1e-7
ROUTINGS = 3

_cache = {}
_NO_CC = False


def _build():
    import concourse.bass as bass
    import concourse.bacc as bacc
    import concourse.mybir as mybir
    import concourse.tile as tile

    f32 = mybir.dt.float32
    bf16 = mybir.dt.bfloat16

    nc = bacc.Bacc("TRN2", target_bir_lowering=False, debug=False)

    XT_d = nc.dram_tensor("xt", [IL, B], bf16, kind="ExternalInput")
    XB_d = nc.dram_tensor("xb", [B, IL], bf16, kind="ExternalInput")
    WB_d = nc.dram_tensor("wb", [IL, JK], bf16, kind="ExternalInput")
    W2_d = nc.dram_tensor("w2", [J, K, IL], bf16, kind="ExternalInput")
    out_d = nc.dram_tensor("v_out", [B, JK], f32, kind="ExternalOutput")

    with tile.TileContext(nc) as tc:
        with (
            tc.tile_pool(name="res", bufs=1) as res,
            tc.tile_pool(name="wbs", bufs=2) as wbs,
            tc.tile_pool(name="w2s", bufs=2) as w2s,
            tc.tile_pool(name="crep", bufs=2) as crepp,
            tc.tile_pool(name="yp", bufs=2) as yp,
            tc.tile_pool(name="prod", bufs=1) as prodp,
            tc.tile_pool(name="ps", bufs=1, space="PSUM") as ps,
            tc.tile_pool(name="ptp", bufs=4, space="PSUM") as ptp,
            tc.tile_pool(name="dram", bufs=2, space="DRAM") as dram,
        ):
            # ---- resident SBUF tensors ----
            XT_s = res.tile([128, NCH * B], bf16)      # chunk-major x^T
            XB_s = res.tile([B, IL], bf16)
            G_s = res.tile([B, J * IS], f32)           # routing logits [b,(j,i)]
            E_s = res.tile([B, J * IS], bf16)          # exp(logits)
            esum_s = res.tile([B, IS], f32)
            R_s = res.tile([B, IS], f32)
            s_loc = res.tile([B, JK], f32)
            s_full = res.tile([B, JK], f32)
            V_cum = res.tile([B, JK], f32)
            V_bf = res.tile([B, JK], bf16)
            V_T4 = res.tile([128, B * J], bf16)        # 4 stacked [k, b*64+j]
            sq = res.tile([B, J], f32)
            d1 = res.tile([B, J], f32)
            d2 = res.tile([B, J], f32)
            d3 = res.tile([B, J], f32)
            rr = res.tile([B, J], f32)
            sc = res.tile([B, J], f32)

            # load residents
            nc.sync.dma_start(
                XT_s[:, :].rearrange("p (c b) -> p c b", b=B),
                XT_d.ap().rearrange("(c p) b -> p c b", p=128),
            )
            nc.sync.dma_start(XB_s[:, :], XB_d[:, :])

            cc_in = dram.tile([B, JK], f32)
            cc_out = dram.tile([B, JK], f32)
            vbuf = dram.tile([B, JK], bf16)
            cbuf = dram.tile([B, J * IS], bf16)

            def all_reduce_s():
                if _NO_CC:
                    nc.vector.tensor_copy(s_full[:, :], s_loc[:, :])
                    return
                nc.sync.dma_start(cc_in[:, :], s_loc[:, :])
                nc.gpsimd.collective_compute(
                    "AllReduce",
                    mybir.AluOpType.add,
                    replica_groups=[list(range(NCORES))],
                    ins=[cc_in.opt()],
                    outs=[cc_out.opt()],
                )
                nc.sync.dma_start(s_full[:, :], cc_out[:, :])

            def squash_and_accum(first):
                # v = s*sq/(1+sq)/sqrt(sq+eps); V_cum += v (v left in s_full)
                nc.scalar.square(s_loc[:, :], s_full[:, :])
                nc.vector.tensor_reduce(
                    sq[:, :],
                    s_loc[:, :].rearrange("b (j k) -> b j k", k=K),
                    axis=mybir.AxisListType.X,
                    op=mybir.AluOpType.add,
                )
                nc.vector.tensor_scalar_add(d1[:, :], sq[:, :], EPS)
                nc.scalar.sqrt(d1[:, :], d1[:, :])
                nc.vector.tensor_scalar_add(d2[:, :], sq[:, :], 1.0)
                nc.vector.tensor_mul(d3[:, :], d1[:, :], d2[:, :])
                nc.vector.reciprocal(rr[:, :], d3[:, :])
                nc.vector.tensor_mul(sc[:, :], sq[:, :], rr[:, :])
                nc.vector.tensor_tensor(
                    out=s_full[:, :],
                    in0=s_full[:, :],
                    in1=sc[:, :].rearrange("b (j o) -> b j o", o=1).broadcast_to([B, J, K]),
                    op=mybir.AluOpType.mult,
                )
                if first:
                    nc.vector.tensor_copy(V_cum[:, :], s_full[:, :])
                else:
                    nc.vector.tensor_add(V_cum[:, :], V_cum[:, :], s_full[:, :])
                # V_bf stored k-major: V_bf[b, k*64+j] = V_cum[b, j*32+k]
                nc.vector.tensor_copy(
                    V_bf[:, :],
                    V_cum[:, :].rearrange("b (j k) -> b k j", k=K),
                )
                # V_T[k, b*64+j] = V_bf[b, k*64+j], via DRAM bounce
                nc.sync.dma_start(vbuf[:, :], V_bf[:, :])
                for t in range(4):
                    nc.sync.dma_start(
                        V_T4[t * K:(t + 1) * K, :].rearrange(
                            "k (b j) -> k b j", j=J
                        ),
                        vbuf[:, :].rearrange("b (k j) -> k b j", k=K),
                    )

            # ======== iteration 0: c = 1/J ========
            ps0 = ps.tile([128, JK], f32, tag="ps")
            for ch in range(NCH):
                wbt = wbs.tile([128, JK], bf16)
                nc.sync.dma_start(wbt[:, :], WB_d[ch * 128:(ch + 1) * 128, :])
                for q in range(4):
                    nc.tensor.matmul(
                        ps0[:B, q * 512:(q + 1) * 512],
                        XT_s[:, ch * B:(ch + 1) * B],
                        wbt[:, q * 512:(q + 1) * 512],
                        start=(ch == 0),
                        stop=(ch == NCH - 1),
                    )
            nc.scalar.mul(s_loc[:, :], ps0[:B, :], 1.0 / J)
            all_reduce_s()
            squash_and_accum(first=True)

            # ======== iterations 1..2 ========
            for r in range(1, ROUTINGS):
                # ---- t-pass + g: logits G[b,(j,i)] = sum_k V.u_hat ----
                # 4 j's row-packed via tile_position: each 32-row strip of
                # the PE array runs an independent k=32-contraction matmul
                for jg in range(J // 4):
                    w2t = w2s.tile([128, IL], bf16, tag="w2t")
                    nc.sync.dma_start(
                        w2t[:, :],
                        W2_d[jg * 4:(jg + 1) * 4, :, :].rearrange(
                            "a k il -> (a k) il"
                        ),
                    )
                    for q in range(8):
                        pts = []
                        for t in range(4):
                            j = jg * 4 + t
                            pt = ptp.tile([B, 512], f32, tag="pt")
                            nc.tensor.matmul(
                                pt[:, :],
                                V_T4[t * K:(t + 1) * K, :].rearrange(
                                    "k (b j) -> k b j", j=J
                                )[:, :, j],
                                w2t[t * K:(t + 1) * K,
                                    q * 512:(q + 1) * 512],
                                start=True,
                                stop=True,
                                tile_position=(t * K, 0),
                            )
                            pts.append(pt)
                        for t in range(4):
                            j = jg * 4 + t
                            prod = prodp.tile([B, 512], f32, tag="prod")
                            nc.vector.tensor_tensor(
                                out=prod[:, :],
                                in0=pts[t][:, :],
                                in1=XB_s[:, q * 512:(q + 1) * 512],
                                op=mybir.AluOpType.mult,
                            )
                            nc.vector.tensor_reduce(
                                G_s[:, j * IS + q * 32: j * IS + (q + 1) * 32],
                                prod[:, :].rearrange("b (i l) -> b i l", l=L),
                                axis=mybir.AxisListType.X,
                                op=mybir.AluOpType.add,
                            )
                # ---- softmax over j ----
                nc.scalar.activation(
                    G_s[:, :], G_s[:, :], mybir.ActivationFunctionType.Exp
                )
                nc.vector.tensor_reduce(
                    esum_s[:, :],
                    G_s[:, :].rearrange("b (j i) -> b i j", j=J),
                    axis=mybir.AxisListType.X,
                    op=mybir.AluOpType.add,
                )
                nc.vector.reciprocal(R_s[:, :], esum_s[:, :])
                nc.vector.tensor_tensor(
                    out=E_s[:, :],
                    in0=G_s[:, :],
                    in1=R_s[:, :]
                    .rearrange("b (o i) -> b o i", o=1)
                    .broadcast_to([B, J, IS]),
                    op=mybir.AluOpType.mult,
                )
                # ---- s-pass: s[b,(j,k)] = sum_il (c*x) . WB ----
                # rows are l-major: chunk ch = (l=ch>>1, i-half=ch&1)
                nc.sync.dma_start(cbuf[:, :], E_s[:, :])
                creps = []
                for ci in range(2):
                    crep = crepp.tile([128, B * J], bf16, tag="crep")
                    nc.sync.dma_start(
                        crep[:, :].rearrange("p (b j) -> p b j", j=J),
                        cbuf[:, :].rearrange("b (j i) -> i b j", j=J)[
                            ci * 128:(ci + 1) * 128
                        ],
                    )
                    creps.append(crep)
                pss = ps.tile([128, JK], f32, tag="ps")
                for ch in range(NCH):
                    # write Y in (j,b) order so s-matmul stationaries are
                    # contiguous; inputs keep their (b,j)/broadcast layouts
                    yt = yp.tile([128, J * B], bf16)
                    nc.vector.tensor_tensor(
                        out=yt[:, :].rearrange("p (j b) -> p j b", j=J),
                        in0=creps[ch % 2][:, :].rearrange(
                            "p (b j) -> p j b", j=J
                        ),
                        in1=XT_s[:, ch * B:(ch + 1) * B]
                        .rearrange("p (o b) -> p o b", o=1)
                        .broadcast_to([128, J, B]),
                        op=mybir.AluOpType.mult,
                    )
                    wbt = wbs.tile([128, JK], bf16)
                    nc.sync.dma_start(
                        wbt[:, :], WB_d[ch * 128:(ch + 1) * 128, :]
                    )
                    for j in range(J):
                        # one start=True per PSUM bank (16 j-slices/bank):
                        # its bank-wide has_written clear must precede all
                        # other j's writes, which land with start=False
                        nc.tensor.matmul(
                            pss[:B, j * K:(j + 1) * K],
                            yt[:, j * B:(j + 1) * B],
                            wbt[:, j * K:(j + 1) * K],
                            start=(ch == 0 and j % 16 == 0),
                            stop=(ch == NCH - 1),
                            skip_group_check=True,
                        )
                nc.vector.tensor_copy(s_loc[:, :], pss[:B, :])
                all_reduce_s()
                squash_and_accum(first=False)

            # s_full now holds v_2 = output
            nc.sync.dma_start(out_d[:, :], s_full[:, :])

    nc.finalize()
    return nc


def _make_in_maps(inputs, W):
    import ml_dtypes

    bf = ml_dtypes.bfloat16
    in_maps = []
    for c in range(NCORES):
        xs = inputs[:, c * IS:(c + 1) * IS, :].astype(np.float32)
        Ws = W[:, c * IS:(c + 1) * IS, :, :].astype(np.float32)
        XT = np.ascontiguousarray(
            xs.transpose(2, 1, 0).reshape(IL, B)
        ).astype(bf)
        XB = np.ascontiguousarray(xs.reshape(B, IL)).astype(bf)
        WB = np.ascontiguousarray(
            Ws.transpose(3, 1, 0, 2).reshape(IL, JK)
        ).astype(bf)
        W2 = np.ascontiguousarray(
            Ws.transpose(0, 2, 1, 3).reshape(J, K, IL)
        ).astype(bf)
        in_maps.append({"xt": XT, "xb": XB, "wb": WB, "w2": W2})
    return in_maps


def kernel(inputs, W):
    from concourse.bass_utils import run_bass_kernel_spmd

    if "nc" not in _cache:
        _cache["nc"] = _build()
    nc = _cache["nc"]

    in_maps = _make_in_maps(inputs, W)
    _cache["in_maps"] = in_maps
    globals()["_last_in_maps"] = in_maps
    res = run_bass_kernel_spmd(nc, in_maps, core_ids=list(range(NCORES)))
    v = res.results[0]["v_out"]
    return np.asarray(v, dtype=np.float32).reshape(B, J, K)



# revision 9
# speedup vs baseline: 2.6031x; 2.6031x over previous
"""CapsuleLayer dynamic-routing kernel for 8x trn2 NeuronCores.

Sharding: input-capsule axis i (2048) split 8 ways (256/core). Routing
softmax over j is core-local; per-iteration s = sum_i c*u_hat is partial
per core, finished with an 8-core AllReduce. u_hat is never materialized.

Key layout choices (vs the 64-partition baseline):
  * All big DVE ops run on 128 partitions. The t-pass packs two j's per
    PSUM tile via tile_position (out partitions = (j&1, b)); the s-pass
    packs (j//32, b).
  * t-pass pipeline per [128,512] unit: PE matmul -> Act copy (PSUM f32
    -> SBUF bf16) -> DVE mult (bf16 2x_1p) -> reduce over l, split
    between DVE and GPSIMD.
  * V^T for the t-pass stationaries comes from PE transposes (identity
    matmul), not a DRAM bounce.
  * softmax normalization is folded into per-chunk XR = x^T * (1/esum),
    so the s-pass Y formation is crep (x) XR with all-bf16 packed
    operands (2x_1p).

Per-core layouts (i-shard, IS=256, IL=IS*L=4096):
  XT  [IL, B]      x^T, rows l*IS+i           (pass-0 stationary, XR)
  XB2 [128, IL]    x[b,(i,l)] dup'd js twice  (t-pass elementwise)
  WB  [IL, J*K]    W as [(l,i),(j,k)]         (pass-0 + s-pass moving)
  W2  [J, K, IL]   W as [j,k,(i,l)]           (t-pass moving)
  ident [128,64]   two stacked 64x64 identities (PE transpose)

On-chip logit layout: G[(j&1)*64+b, (j>>1)*256 + i]  (f32)
On-chip s/v layout:   s2[(j//32)*64+b, (j%32)*32 + k] (f32)
"""

import numpy as np

B, I_FULL, L = 64, 2048, 16
J, K = 64, 32
NCORES = 8
IS = I_FULL // NCORES          # 256 i per core
IL = IS * L                    # 4096
NCH = IL // 128                # 32 contraction chunks of 128
JK = J * K                     # 2048
EPS = 1e-7
ROUTINGS = 3

# fraction of t-pass reduces sent to GPSIMD (Pool engine)
GPS_REDUCE_MOD = 3  # unit % 3 != 0 -> GPSIMD (2/3 of reduces)

_cache = {}


def _build():
    import concourse.bass as bass
    import concourse.bacc as bacc
    import concourse.mybir as mybir
    import concourse.tile as tile

    f32 = mybir.dt.float32
    bf16 = mybir.dt.bfloat16

    nc = bacc.Bacc("TRN2", target_bir_lowering=False, debug=False)

    XT_d = nc.dram_tensor("xt", [IL, B], bf16, kind="ExternalInput")
    XB2_d = nc.dram_tensor("xb2", [128, IL], bf16, kind="ExternalInput")
    WB_d = nc.dram_tensor("wb", [IL, JK], bf16, kind="ExternalInput")
    W2_d = nc.dram_tensor("w2", [J, K, IL], bf16, kind="ExternalInput")
    ID_d = nc.dram_tensor("ident", [128, 64], bf16, kind="ExternalInput")
    out_d = nc.dram_tensor("v_out", [B, JK], f32, kind="ExternalOutput")

    with tile.TileContext(nc) as tc:
        with (
            tc.tile_pool(name="res", bufs=1) as res,
            tc.tile_pool(name="wbs", bufs=2) as wbs,
            tc.tile_pool(name="w2s", bufs=2) as w2s,
            tc.tile_pool(name="yp", bufs=2) as yp,
            tc.tile_pool(name="pin", bufs=3) as pinp,
            tc.tile_pool(name="prod", bufs=3) as prodp,
            tc.tile_pool(name="ps", bufs=1, space="PSUM") as ps,      # 2 banks
            tc.tile_pool(name="ptp", bufs=4, space="PSUM") as ptp,    # 4 banks
            tc.tile_pool(name="ptr", bufs=2, space="PSUM") as ptrp,   # 2 banks
            tc.tile_pool(name="dram", bufs=1, space="DRAM") as dram,
        ):
            # ---- resident SBUF tensors ----
            XT_s = res.tile([128, NCH * B], bf16)      # chunk-major x^T
            XB2_s = res.tile([128, IL], bf16)
            XR_s = res.tile([128, NCH * B], bf16)      # x^T * R per chunk
            ID_s = res.tile([128, 64], bf16)
            G_s = res.tile([128, J // 2 * IS], f32)    # [ (j&1,b), (j>>1, i) ]
            E_s = res.tile([128, J // 2 * IS], bf16)
            crep0 = res.tile([128, J * B], bf16)       # [i(0:128), (j, b)]
            crep1 = res.tile([128, J * B], bf16)       # [i(128:256), (j, b)]
            esum = res.tile([128, 2 * B], f32)         # per ci halves
            R_f = res.tile([128, 2 * B], f32)
            R_bf = res.tile([128, 2 * B], bf16)
            s_loc = res.tile([128, J // 2 * K], f32)   # [(j//32,b),(j%32,k)]
            s_full = res.tile([128, J // 2 * K], f32)
            V_cum = res.tile([128, J // 2 * K], f32)
            V_bf = res.tile([128, J // 2 * K], bf16)
            VT_s = res.tile([128, 16 * B], bf16)       # 16 x [ (t,k), b ]
            sq = res.tile([128, J // 2], f32)
            d1 = res.tile([128, J // 2], f32)
            d3 = res.tile([128, J // 2], f32)
            rr = res.tile([128, J // 2], f32)
            sc = res.tile([128, J // 2], f32)

            # load residents
            nc.sync.dma_start(
                XT_s[:, :].rearrange("p (c b) -> p c b", b=B),
                XT_d.ap().rearrange("(c p) b -> p c b", p=128),
            )
            nc.sync.dma_start(XB2_s[:, :], XB2_d[:, :])
            nc.sync.dma_start(ID_s[:, :], ID_d[:, :])

            cc_in = dram.tile([B, JK], f32)
            cc_out = dram.tile([B, JK], f32)

            creps = [crep0, crep1]

            def all_reduce_s():
                # s_loc [(js,b),(p,k)] -> cc_in[b, (js,p,k)] (4KB rows)
                for js in range(2):
                    nc.sync.dma_start(
                        cc_in[:, js * 1024:(js + 1) * 1024],
                        s_loc[js * 64:(js + 1) * 64, :],
                    )
                nc.gpsimd.collective_compute(
                    "AllReduce",
                    mybir.AluOpType.add,
                    replica_groups=[list(range(NCORES))],
                    ins=[cc_in.opt()],
                    outs=[cc_out.opt()],
                )
                for js in range(2):
                    nc.sync.dma_start(
                        s_full[js * 64:(js + 1) * 64, :],
                        cc_out[:, js * 1024:(js + 1) * 1024],
                    )

            def squash_and_accum(first):
                # v = s*sq/(1+sq)/sqrt(sq+eps); V_cum += v (v left in s_full)
                P2 = J // 2  # 32 j-slices per partition-half
                nc.scalar.square(s_loc[:, :], s_full[:, :])
                nc.vector.tensor_reduce(
                    sq[:, :],
                    s_loc[:, :].rearrange("P (p k) -> P p k", k=K),
                    axis=mybir.AxisListType.X,
                    op=mybir.AluOpType.add,
                )
                nc.vector.tensor_scalar_add(d1[:, :], sq[:, :], EPS)
                nc.scalar.sqrt(d1[:, :], d1[:, :])
                nc.vector.tensor_scalar_add(d3[:, :], sq[:, :], 1.0)
                nc.vector.tensor_mul(d3[:, :], d1[:, :], d3[:, :])
                nc.vector.reciprocal(rr[:, :], d3[:, :])
                nc.vector.tensor_mul(sc[:, :], sq[:, :], rr[:, :])
                nc.vector.tensor_tensor(
                    out=s_full[:, :],
                    in0=s_full[:, :],
                    in1=sc[:, :].rearrange("P (p o) -> P p o", o=1)
                    .broadcast_to([128, P2, K]),
                    op=mybir.AluOpType.mult,
                )
                if first:
                    nc.vector.tensor_copy(V_cum[:, :], s_full[:, :])
                else:
                    nc.vector.tensor_add(V_cum[:, :], V_cum[:, :], s_full[:, :])
                nc.vector.tensor_copy(V_bf[:, :], V_cum[:, :])
                # V^T via PE transposes: for jg group g (j=4g..4g+3):
                # in = V_bf[js2-half, (p2 in 4g%32..+4, k)]  [64, 128]
                # out = [ (t,k), b ] -> VT_s[:, g*64:(g+1)*64]
                for g in range(16):
                    js2 = g // 8
                    c0 = (4 * g) % 32 * K
                    ptt = ptrp.tile([128, B], bf16, tag="ptt")
                    nc.tensor.transpose(
                        ptt[:, :],
                        V_bf[js2 * 64:(js2 + 1) * 64, c0:c0 + 128],
                        ID_s[js2 * 64:(js2 + 1) * 64, :],
                    )
                    nc.scalar.copy(VT_s[:, g * B:(g + 1) * B], ptt[:, :])

            # ======== pass 0: s0 = (1/J) * sum_i u_hat ========
            ps0 = ps.tile([128, J // 2 * K], f32, tag="ps")
            for ch in range(NCH):
                wbt = wbs.tile([128, JK], bf16)
                nc.sync.dma_start(wbt[:, :], WB_d[ch * 128:(ch + 1) * 128, :])
                for js2 in range(2):
                    for h in range(2):
                        nc.tensor.matmul(
                            ps0[js2 * 64:(js2 + 1) * 64,
                                h * 512:(h + 1) * 512],
                            XT_s[:, ch * B:(ch + 1) * B],
                            wbt[:, js2 * 1024 + h * 512:
                                js2 * 1024 + (h + 1) * 512],
                            start=(ch == 0),
                            stop=(ch == NCH - 1),
                            tile_position=(0, js2 * 64),
                            skip_group_check=True,
                        )
            nc.scalar.mul(s_loc[:, :], ps0[:, :], 1.0 / J)
            all_reduce_s()
            squash_and_accum(first=True)

            # ======== iterations 1..2 ========
            for r in range(1, ROUTINGS):
                # ---- t-pass: G[(j&1)b, (j>>1)i] = sum_k v.u_hat ----
                unit = 0
                for g in range(16):
                    w2t = w2s.tile([128, IL], bf16, tag="w2t")
                    nc.sync.dma_start(
                        w2t[:, :],
                        W2_d[g * 4:(g + 1) * 4, :, :].rearrange(
                            "a k il -> (a k) il"
                        ),
                    )
                    for q in range(8):
                        for a in range(2):  # pair a: j = 4g+2a, 4g+2a+1
                            pr = 2 * g + a
                            pt = ptp.tile([128, 512], f32, tag="pt")
                            for t in (2 * a, 2 * a + 1):
                                js = t & 1
                                nc.tensor.matmul(
                                    pt[js * 64:(js + 1) * 64, :],
                                    VT_s[t * K:(t + 1) * K,
                                         g * B:(g + 1) * B],
                                    w2t[t * K:(t + 1) * K,
                                        q * 512:(q + 1) * 512],
                                    start=True,
                                    stop=True,
                                    tile_position=(t * K, js * 64),
                                    skip_group_check=True,
                                )
                            prod = prodp.tile([128, 512], bf16, tag="prod")
                            if unit % 16 < 3:
                                # Act copy PSUM->SBUF bf16, DVE 2x_1p mult
                                pin = pinp.tile([128, 512], bf16, tag="pin")
                                nc.scalar.copy(pin[:, :], pt[:, :])
                                nc.vector.tensor_tensor(
                                    out=prod[:, :],
                                    in0=pin[:, :],
                                    in1=XB2_s[:, q * 512:(q + 1) * 512],
                                    op=mybir.AluOpType.mult,
                                )
                            else:
                                # GPSIMD fused mult straight from PSUM
                                nc.gpsimd.scalar_tensor_tensor(
                                    out=prod[:, :],
                                    in0=pt[:, :],
                                    scalar=1.0,
                                    in1=XB2_s[:, q * 512:(q + 1) * 512],
                                    op0=mybir.AluOpType.mult,
                                    op1=mybir.AluOpType.mult,
                                )
                            gout = G_s[:, pr * IS + q * 32: pr * IS + (q + 1) * 32]
                            gin = prod[:, :].rearrange("P (i l) -> P i l", l=L)
                            nc.vector.tensor_reduce(
                                gout, gin,
                                axis=mybir.AxisListType.X,
                                op=mybir.AluOpType.add,
                            )
                            unit += 1
                    # exp + SBUF->SBUF DMA-transpose into crep
                    for a in range(2):
                        pr = 2 * g + a
                        nc.scalar.activation(
                            E_s[:, pr * IS:(pr + 1) * IS],
                            G_s[:, pr * IS:(pr + 1) * IS],
                            mybir.ActivationFunctionType.Exp,
                        )
                        for ci in range(2):
                            nc.sync.dma_start_transpose(
                                creps[ci][:, pr * 128:(pr + 1) * 128],
                                E_s[:, pr * IS + ci * 128:
                                    pr * IS + (ci + 1) * 128],
                            )

                # ---- softmax denominators from transposed E ----
                for ci in range(2):
                    # esum over j: crep [p, (j, b)] -> [p, b]
                    nc.vector.tensor_reduce(
                        esum[:, ci * B:(ci + 1) * B],
                        creps[ci][:, :].rearrange("p (j b) -> p b j", j=J),
                        axis=mybir.AxisListType.X,
                        op=mybir.AluOpType.add,
                    )
                    nc.vector.reciprocal(
                        R_f[:, ci * B:(ci + 1) * B],
                        esum[:, ci * B:(ci + 1) * B],
                    )
                    nc.vector.tensor_copy(
                        R_bf[:, ci * B:(ci + 1) * B],
                        R_f[:, ci * B:(ci + 1) * B],
                    )

                # ---- s-pass ----
                # chunk ch rows: l = ch//2, i-half ci = ch%2
                pss = ps.tile([128, J // 2 * K], f32, tag="ps")
                for ci in range(2):
                    for lh in range(L):
                        ch = lh * 2 + ci
                        # XR[p, b] = x^T[p, (ch,b)] * R[p, (ci,b)]
                        nc.vector.tensor_tensor(
                            out=XR_s[:, ch * B:(ch + 1) * B],
                            in0=XT_s[:, ch * B:(ch + 1) * B],
                            in1=R_bf[:, ci * B:(ci + 1) * B],
                            op=mybir.AluOpType.mult,
                        )
                        yt = yp.tile([128, J * B], bf16)
                        nc.vector.tensor_tensor(
                            out=yt[:, :].rearrange("p (j b) -> p j b", j=J),
                            in0=creps[ci][:, :].rearrange(
                                "p (j b) -> p j b", j=J
                            ),
                            in1=XR_s[:, ch * B:(ch + 1) * B]
                            .rearrange("p (o b) -> p o b", o=1)
                            .broadcast_to([128, J, B]),
                            op=mybir.AluOpType.mult,
                        )
                        wbt = wbs.tile([128, JK], bf16)
                        nc.sync.dma_start(
                            wbt[:, :], WB_d[ch * 128:(ch + 1) * 128, :]
                        )
                        first_ch = ci == 0 and lh == 0
                        for j in range(J):
                            js2, p2 = j // 32, j % 32
                            nc.tensor.matmul(
                                pss[js2 * 64:(js2 + 1) * 64,
                                    p2 * K:(p2 + 1) * K],
                                yt[:, j * B:(j + 1) * B],
                                wbt[:, j * K:(j + 1) * K],
                                start=(first_ch and j % 16 == 0),
                                stop=(ci == 1 and lh == L - 1),
                                tile_position=(0, js2 * 64),
                                skip_group_check=True,
                            )
                nc.vector.tensor_copy(s_loc[:, :], pss[:, :])
                all_reduce_s()
                squash_and_accum(first=False)

            # s_full now holds v_2; unscramble [(js,b),(p,k)] -> [b, j*K+k]
            for js in range(2):
                nc.sync.dma_start(
                    out_d[:, js * 1024:(js + 1) * 1024],
                    s_full[js * 64:(js + 1) * 64, :],
                )

    nc.finalize()
    return nc


def _make_in_maps(inputs, W):
    import ml_dtypes

    bf = ml_dtypes.bfloat16
    ident = np.zeros((128, 64), dtype=np.float32)
    ident[:64] = np.eye(64)
    ident[64:] = np.eye(64)
    ident = ident.astype(bf)
    in_maps = []
    for c in range(NCORES):
        xs = inputs[:, c * IS:(c + 1) * IS, :].astype(np.float32)
        Ws = W[:, c * IS:(c + 1) * IS, :, :].astype(np.float32)
        XT = np.ascontiguousarray(
            xs.transpose(2, 1, 0).reshape(IL, B)
        ).astype(bf)
        XB = np.ascontiguousarray(xs.reshape(B, IL)).astype(bf)
        XB2 = np.concatenate([XB, XB], axis=0)  # [128, IL]
        WB = np.ascontiguousarray(
            Ws.transpose(3, 1, 0, 2).reshape(IL, JK)
        ).astype(bf)
        W2 = np.ascontiguousarray(
            Ws.transpose(0, 2, 1, 3).reshape(J, K, IL)
        ).astype(bf)
        in_maps.append(
            {"xt": XT, "xb2": XB2, "wb": WB, "w2": W2, "ident": ident}
        )
    return in_maps


def kernel(inputs, W):
    from concourse.bass_utils import run_bass_kernel_spmd

    if "nc" not in _cache:
        _cache["nc"] = _build()
    nc = _cache["nc"]

    in_maps = _make_in_maps(inputs, W)
    _cache["in_maps"] = in_maps
    globals()["_last_in_maps"] = in_maps
    res = run_bass_kernel_spmd(nc, in_maps, core_ids=list(range(NCORES)))
    v = res.results[0]["v_out"]
    out = np.asarray(v, dtype=np.float32).reshape(B, JK)
    # rows of v_out are [b, (js,p,k)] with j = js*32+p -> already j*K+k. The
    # on-chip layout was [(js,b),(p,k)] but the output DMA unscrambled it.
    return out.reshape(B, J, K)
